# revision 8
# baseline (speedup 1.0000x reference)
"""CRF forward-algorithm kernel for 8 Trainium2 NeuronCores.

Contract: kernel(**inputs) takes the FULL unsharded inputs (numpy, f32),
shards batch across 8 cores (data-parallel), runs one SPMD Bass/Tile
program per core, and returns (log_partition [B], alpha [B, T, K]) like
the reference.

Math (per batch row b):
  mu, sig = stats of features[b] over T (ddof=1)
  sf      = clip((x-mu)/sig @ W.T + bias, -5, 5)          [T, K]
          = clip(x @ (W/sig).T + (bias - (W/sig)@mu), ...)  (weights
            normalized instead of the features -> no elementwise pass)
  trans   = clip(Wt + bt, -5, 5);  E = exp(trans)
  alpha[t] = logsumexp_j(alpha[t-1, j] + trans[j, k]) + sf[t]
  Computed in the exp domain (scaled HMM forward):
     S[t] = (E^T @ S[t-1]) * exp(sf[t]) / (per-b rescale every R steps)
     alpha[t] = ln(S[t]) + C[epoch(t)]
  The matmul's extra ones-column supplies the rescale mass.

Device layout: state S is [K=48 partitions, b free]; alpha history is
written [K, T, B_local] and the host transposes it back (layout prep is
host-side, like the sharding itself).
"""

import os
import sys

if "/opt/trn_rl_repo" not in sys.path:
    sys.path.insert(0, "/opt/trn_rl_repo")

import numpy as np

B, T, FD, K = 256, 512, 1024, 48
NCORES = 8
BL = B // NCORES          # 32 batch rows per core
NCH = FD // 128           # 8 feature chunks of 128 partitions
G = int(os.environ.get("CRF_G", "2"))    # scan batch groups (pipelining)
W = BL // G               # free width per scan group
R = int(os.environ.get("CRF_R", "8"))    # renorm every R steps
NEP = T // R              # epochs
SPLIT = int(os.environ.get("CRF_SPLIT", "3"))  # chunks on DVE bn_stats; rest ACT/GPS

assert T % R == 0 and BL % G == 0

_cache = {}


def _build():
    """Build + compile the single-core SPMD program. Cached per process."""
    if "nc" in _cache:
        return _cache["nc"]

    from contextlib import ExitStack

    import concourse.bacc as bacc
    import concourse.tile as tile
    from concourse import mybir

    F32 = mybir.dt.float32
    F32R = mybir.dt.float32r
    AF = mybir.ActivationFunctionType
    OP = mybir.AluOpType

    E5 = float(np.exp(np.float32(5.0)))
    EM5 = float(np.exp(np.float32(-5.0)))

    nc = bacc.Bacc("TRN2", target_bir_lowering=False, debug=False, num_devices=1)

    xt_d = nc.dram_tensor("xt", [BL, FD, T], F32, kind="ExternalInput").ap()
    wt_d = nc.dram_tensor("wt", [FD, K], F32, kind="ExternalInput").ap()
    sb_d = nc.dram_tensor("sb", [K, 1], F32, kind="ExternalInput").ap()
    tw_d = nc.dram_tensor("tw", [K, K], F32, kind="ExternalInput").ap()
    tb_d = nc.dram_tensor("tb", [K, K], F32, kind="ExternalInput").ap()
    alpha_d = nc.dram_tensor("alpha", [K, T, BL], F32, kind="ExternalOutput").ap()
    logz_d = nc.dram_tensor("logz", [1, BL], F32, kind="ExternalOutput").ap()

    with tile.TileContext(nc) as tc, ExitStack() as ctx:
        pc = ctx.enter_context(tc.tile_pool(name="consts", bufs=1))
        px = ctx.enter_context(tc.tile_pool(name="x", bufs=2))
        pg = ctx.enter_context(tc.tile_pool(name="g", bufs=2))
        pst = ctx.enter_context(tc.tile_pool(name="stats", bufs=2))
        pscr = ctx.enter_context(tc.tile_pool(name="scratch", bufs=2))
        pS = ctx.enter_context(tc.tile_pool(name="scan", bufs=4))
        psm = ctx.enter_context(tc.tile_pool(name="small", bufs=4))
        psf = ctx.enter_context(tc.tile_pool(name="psum_sf", bufs=2, space="PSUM"))
        pd = ctx.enter_context(tc.tile_pool(name="psum_d", bufs=2, space="PSUM"))
        pmm = ctx.enter_context(tc.tile_pool(name="psum_mm", bufs=2, space="PSUM"))

        # ---------------- constants ----------------
        wT = pc.tile([128, NCH * K], F32)          # W^T chunks [128f, 48k] each
        nc.sync.dma_start(wT[:].rearrange("p (c k) -> p c k", c=NCH),
                          wt_d.rearrange("(c p) k -> p c k", p=128))
        sb_sb = pc.tile([K, 1], F32)
        nc.sync.dma_start(sb_sb[:], sb_d[:])
        tw_sb = pc.tile([K, K], F32)
        nc.sync.dma_start(tw_sb[:], tw_d[:])
        tb_sb = pc.tile([K, K], F32)
        nc.sync.dma_start(tb_sb[:], tb_d[:])

        # E1 = [exp(clip(tw+tb, +-5)) | 0-pad | ones] in [K, 65]: the
        # ones column sits at 64 so the mass row lands on a 32-aligned
        # PSUM partition (engine reads must start at 0/32/64/96).
        EW = 65
        E1 = pc.tile([K, EW], F32)
        ttmp = pc.tile([K, K], F32)
        nc.vector.tensor_tensor(ttmp[:], tw_sb[:], tb_sb[:], OP.add)
        nc.vector.tensor_scalar(ttmp[:], ttmp[:], -5.0, 5.0, OP.max, OP.min)
        nc.scalar.activation(E1[:, 0:K], ttmp[:], AF.Exp)
        nc.gpsimd.memset(E1[:, K:EW - 1], 0.0)
        nc.gpsimd.memset(E1[:, EW - 1:EW], 1.0)
        ones_col = pc.tile([K, 1], F32)
        nc.gpsimd.memset(ones_col[:], 1.0)

        es_all = pc.tile([K, BL * T], F32)         # exp(sf), layout [k, (b, t)]
        hist = pc.tile([K, T * BL], F32)           # ln(S), layout [k, (t, b)]
        C_hist = pc.tile([1, NEP * BL], F32)       # per-epoch offsets [(e, b)]
        logz_sb = pc.tile([1, BL], F32)
        nc.gpsimd.memset(C_hist[:, 0:BL], 0.0)

        # ---------------- phase 1: sf -> es per batch row ----------------
        sc_var = float(T) / float(T - 1)           # ddof=1 correction
        for b in range(BL):
            xt = px.tile([128, NCH * T], F32R)
            nc.sync.dma_start(xt[:].rearrange("p (c t) -> p c t", c=NCH),
                              xt_d[b].bitcast(F32R).rearrange("(c p) t -> p c t", p=128))

            st6 = pst.tile([128, 6 * max(SPLIT, 1)], F32)
            aggr = pst.tile([128, 2 * max(SPLIT, 1)], F32)
            mu_a = pst.tile([128, NCH], F32)       # per-chunk means
            sq_a = pst.tile([128, NCH], F32)       # ACT-chunk sum(x^2)
            m2s = pst.tile([128, NCH], F32)
            sig2 = pst.tile([128, NCH], F32)
            sig = pst.tile([128, NCH], F32)
            rsig = pst.tile([128, NCH], F32)

            for c in range(NCH):
                xc = xt[:, c * T:(c + 1) * T].bitcast(F32)
                if c < SPLIT:
                    # one-pass mean+var on DVE
                    nc.vector.bn_stats(st6[:, c * 6:(c + 1) * 6], xc)
                    nc.vector.bn_aggr(aggr[:, 2 * c:2 * c + 2], st6[:, c * 6:(c + 1) * 6])
                    nc.vector.tensor_scalar_mul(sig2[:, c:c + 1],
                                                aggr[:, 2 * c + 1:2 * c + 2], sc_var)
                    nc.vector.tensor_copy(mu_a[:, c:c + 1], aggr[:, 2 * c:2 * c + 1])
                else:
                    # sum(x^2) on ACT, sum(x) on DVE (tensor_scalar runs 2x)
                    scr = pscr.tile([128, T], F32)
                    nc.scalar.activation(scr[:], xc, AF.Square,
                                         accum_out=sq_a[:, c:c + 1])
                    scr2 = pscr.tile([128, T], F32)
                    nc.vector.tensor_scalar(scr2[:], xc, 1.0, 0.0, OP.mult,
                                            OP.add, accum_out=mu_a[:, c:c + 1])
                    nc.vector.tensor_scalar_mul(mu_a[:, c:c + 1], mu_a[:, c:c + 1],
                                                1.0 / float(T))
                    nc.scalar.activation(m2s[:, c:c + 1], mu_a[:, c:c + 1], AF.Square,
                                         scale=float(np.sqrt(sc_var)))
                    # sig2 = sq/(T-1) - mu^2*T/(T-1)
                    nc.vector.scalar_tensor_tensor(sig2[:, c:c + 1], sq_a[:, c:c + 1],
                                                   1.0 / float(T - 1), m2s[:, c:c + 1],
                                                   OP.mult, OP.subtract)
            nc.scalar.activation(sig[:], sig2[:], AF.Sqrt)
            nc.vector.reciprocal(rsig[:], sig[:])

            # normalized weights gT[f, k] = wT[f, k] / sig[f]
            gT = pg.tile([128, NCH * K], F32R)
            for c in range(NCH):
                nc.scalar.activation(gT[:, c * K:(c + 1) * K], wT[:, c * K:(c + 1) * K],
                                     AF.Copy, scale=rsig[:, c:c + 1])

            # d[k] = sum_f mu[f] * gT[f, k]
            dps = pd.tile([K, 1], F32)
            for c in range(NCH):
                nc.tensor.matmul(dps[:], gT[:, c * K:(c + 1) * K].bitcast(F32),
                                 mu_a[:, c:c + 1],
                                 start=(c == 0), stop=(c == NCH - 1))
            bias_b = psm.tile([K, 1], F32)
            nc.vector.tensor_tensor(bias_b[:], sb_sb[:], dps[:], OP.subtract)

            # sf^T = gT.T @ x^T   (fp32r full-rate matmul)
            sfp = psf.tile([K, T], F32)
            for c in range(NCH):
                nc.tensor.matmul(sfp[:], gT[:, c * K:(c + 1) * K],
                                 xt[:, c * T:(c + 1) * T],
                                 start=(c == 0), stop=(c == NCH - 1))
            es_b = es_all[:, b * T:(b + 1) * T]
            nc.scalar.activation(es_b, sfp[:], AF.Exp, bias=bias_b[:])
            nc.gpsimd.tensor_scalar(es_b, es_b, EM5, E5, OP.max, OP.min)

        # ---------------- phase 2: scaled forward scan ----------------
        def es_view(t, g):
            start = g * W * T + t
            return es_all[:, start:start + (W - 1) * T + 1:T]

        S_prev = [None] * G
        for g in range(G):
            S0 = pS.tile([K, W], F32, tag=f"S{g}")
            nc.scalar.copy(S0[:], es_view(0, g))
            nc.scalar.activation(hist[:, g * W:g * W + W], S0[:], AF.Ln)
            S_prev[g] = S0

        for t in range(1, T):
            for g in range(G):
                ps = pmm.tile([EW, W], F32, tag=f"mm{g}")
                nc.tensor.matmul(ps[:], E1[:], S_prev[g][:], start=True, stop=True)
                S_cur = pS.tile([K, W], F32, tag=f"S{g}")
                if t % R == 0:
                    e = t // R
                    rec = psm.tile([1, W], F32, tag=f"rec{g}")
                    nc.vector.reciprocal(rec[:], ps[EW - 1:EW, :])
                    bc = psm.tile([K, W], F32, tag=f"bc{g}")
                    nc.gpsimd.partition_broadcast(bc[:], rec[:])
                    tmp = pS.tile([K, W], F32, tag=f"tmp{g}")
                    nc.vector.tensor_tensor(tmp[:], ps[0:K, :], es_view(t, g), OP.mult)
                    nc.vector.tensor_tensor(S_cur[:], tmp[:], bc[:], OP.mult)
                    lnsg = psm.tile([1, W], F32, tag=f"ln{g}")
                    nc.scalar.activation(lnsg[:], ps[EW - 1:EW, :], AF.Ln)
                    co, cp = e * BL + g * W, (e - 1) * BL + g * W
                    nc.vector.tensor_tensor(C_hist[:, co:co + W],
                                            C_hist[:, cp:cp + W], lnsg[:], OP.add)
                else:
                    nc.vector.tensor_tensor(S_cur[:], ps[0:K, :], es_view(t, g), OP.mult)
                nc.scalar.activation(hist[:, t * BL + g * W:t * BL + g * W + W],
                                     S_cur[:], AF.Ln)
                S_prev[g] = S_cur

        # log partition = ln(sum_k S[T-1]) + C[last]
        for g in range(G):
            pz = pmm.tile([1, W], F32, tag=f"mm{g}")
            nc.tensor.matmul(pz[:], ones_col[:], S_prev[g][:], start=True, stop=True)
            lz = psm.tile([1, W], F32, tag=f"lz{g}")
            nc.scalar.activation(lz[:], pz[:], AF.Ln)
            cl = (NEP - 1) * BL + g * W
            nc.vector.tensor_tensor(logz_sb[:, g * W:g * W + W],
                                    C_hist[:, cl:cl + W], lz[:], OP.add)

        # ---------------- phase 3: alpha = hist + C, DMA out ----------------
        Cb = pc.tile([K, NEP * BL], F32)
        nc.gpsimd.partition_broadcast(Cb[:], C_hist[:])
        hist_v = hist[:].rearrange("p (t b) -> p t b", b=BL)
        cb_v = Cb[:].rearrange("p (e b) -> p e b", b=BL)
        for th in range(R):
            nc.gpsimd.tensor_tensor(hist_v[:, th::R, :], hist_v[:, th::R, :],
                                    cb_v, OP.add)
        nc.sync.dma_start(alpha_d.rearrange("k t b -> k (t b)"), hist[:])
        nc.sync.dma_start(logz_d[:], logz_sb[:])

    nc.compile()
    _cache["nc"] = nc
    return nc


def _prep_inputs(features, state_weights, state_bias, transition_weights,
                 transition_bias):
    features = np.asarray(features, dtype=np.float32)
    w = np.asarray(state_weights, dtype=np.float32)
    sb = np.asarray(state_bias, dtype=np.float32)
    tw = np.asarray(transition_weights, dtype=np.float32)
    tb = np.asarray(transition_bias, dtype=np.float32)

    wt = np.ascontiguousarray(w.T)                      # [FD, K]
    sb2 = np.ascontiguousarray(sb.reshape(K, 1))
    in_maps = []
    for c in range(NCORES):
        sh = features[c * BL:(c + 1) * BL]              # [BL, T, FD]
        xt = np.ascontiguousarray(sh.transpose(0, 2, 1))  # [BL, FD, T]
        in_maps.append({"xt": xt, "wt": wt, "sb": sb2, "tw": tw, "tb": tb})
    return in_maps


def _run(in_maps, trace=False, **kw):
    from concourse.bass_utils import run_bass_kernel_spmd
    nc = _build()
    return run_bass_kernel_spmd(nc, in_maps, core_ids=list(range(NCORES)),
                                trace=trace, **kw)


def _gather(res):
    alphas, logzs = [], []
    for c in range(NCORES):
        a = res.results[c]["alpha"]                     # [K, T, BL]
        alphas.append(np.ascontiguousarray(a.transpose(2, 1, 0)))
        logzs.append(res.results[c]["logz"].reshape(BL))
    return np.concatenate(logzs, axis=0), np.concatenate(alphas, axis=0)


def kernel(features, state_weights, state_bias, transition_weights,
           transition_bias):
    in_maps = _prep_inputs(features, state_weights, state_bias,
                           transition_weights, transition_bias)
    res = _run(in_maps)
    return _gather(res)
